# revision 54
# baseline (speedup 1.0000x reference)
"""Trainium2 Bass kernel for DenseCapsule dynamic routing (3 iterations).

Problem: x[128,2048,8] f32, weight[16,2048,16,8] f32 -> out[128,16,16] f32.
  x_hat = einsum('oide,bie->boid', W, x); 3 routing iterations
  (softmax over o, c-weighted i-sum, squash, agreement update).

Strategy (8 NeuronCores, shard in_num_caps I=2048 -> 256 per core):
  x_hat is never materialized. Per iteration, everything factors through W:
    u = v . W (PE), l = sum_e x*u (DVE), softmax (ACT/DVE),
    xc = c*x (DVE), s = xc @ W (PE).

  v4: iteration 1's c is UNIFORM, so s1 = (1/16) sum_i xhat is fully
  input-determined. Every core loads the FULL W (bf16, host-rotated so its
  own I-slice sits in blocks 0..1) and computes the full s1 locally with
  one 128-matmul chain (overlapped with the W DMA stream) -> NO AllReduce
  for iteration 1. The collectives runtime's ~38us barrier + ~36us
  first-collective init run in the background (~21us in, time-based) and
  are fully amortized by the time AR2 (the only collective) triggers.
  AR2 is split into o-halves so squash/transpose/ul3 of the first half
  overlap the second half's AllReduce.

Layout conventions per core (SBUF partition dim first):
  i_local = ihf*128 + il over the ROTATED I axis (own slice = ihf 0..1)
  o = 4*h + g            (g in 0..3 selects a 32-partition group, h in 0..3)
  d padded to 32 rows (dd) for the u-matmul stationary operand.
"""

import sys

for _p in ("/opt/trn_rl_repo", "/root/.axon_site/_ro/trn_rl_repo"):
    if _p not in sys.path:
        sys.path.insert(0, _p)

import numpy as np
import ml_dtypes

import concourse.bass as bass
import concourse.bacc as bacc
import concourse.mybir as mybir
import concourse.tile as tile
from concourse.bass_utils import run_bass_kernel_spmd

F32 = mybir.dt.float32
BF16 = mybir.dt.bfloat16
F8 = mybir.dt.float8e4
NPBF16 = ml_dtypes.bfloat16
NPF8 = ml_dtypes.float8_e4m3
WS = 64.0  # host pre-scale on fp8 W (keeps values out of fp8 subnormals)

N_CORES = 8
B = 128          # batch
I_FULL = 2048    # in caps
IC = 256         # in caps per core
IL = 128         # partition dim of i
IH = IC // IL    # 2 (own slice blocks)
NIH = I_FULL // IL  # 16 (full-I blocks, rotated: 0..1 are "ours")
E = 8            # in cap dim
O = 16           # out caps
D = 16           # out cap dim
EPS = 1e-8

_CACHE = {}


def _emit_squash_half(nc, pool, sfull_h, vpad, psum_tp, vT, ident, hbase,
                      tag):
    """squash on one o-half: sfull_h [(b)=128, (2h,g,d)=128] f32 covering
    o = 4*hbase .. 4*hbase+7; writes v into vpad cols for h = hbase,
    hbase+1 and produces the two vT column blocks via PE transposes."""
    HO = 8  # o's in this half
    sq = pool.tile([B, HO * D], F32, tag=f"sq{tag}")
    nc.scalar.square(sq[:, :], sfull_h[:, :])
    nrm2 = pool.tile([B, HO], F32, tag=f"nrm2{tag}")
    nc.vector.reduce_sum(
        nrm2[:, :],
        sq[:, :].rearrange("p (o d) -> p o d", d=D),
        axis=mybir.AxisListType.X,
    )
    q = pool.tile([B, HO], F32, tag=f"q{tag}")
    nc.scalar.sqrt(q[:, :], nrm2[:, :])
    t1 = pool.tile([B, HO], F32, tag=f"t1{tag}")
    nc.vector.tensor_scalar_add(t1[:, :], nrm2[:, :], 1.0)
    t2 = pool.tile([B, HO], F32, tag=f"t2{tag}")
    nc.vector.tensor_scalar_add(t2[:, :], q[:, :], EPS)
    den = pool.tile([B, HO], F32, tag=f"den{tag}")
    nc.vector.tensor_mul(den[:, :], t1[:, :], t2[:, :])
    rden = pool.tile([B, HO], F32, tag=f"rden{tag}")
    nc.vector.reciprocal(rden[:, :], den[:, :])
    scale = pool.tile([B, HO], F32, tag=f"scale{tag}")
    nc.vector.tensor_mul(scale[:, :], nrm2[:, :], rden[:, :])
    # v = s * scale (broadcast over d) into vpad[(b), (hh, g, dd<16)]
    s_v = sfull_h[:, :].rearrange("p (hh g d) -> p hh g d", hh=2, g=4)
    scale_v = scale[:, :].rearrange("p (hh g) -> p hh g", hh=2).broadcast_to(
        (B, 2, 4, D)
    )
    vslice = vpad[:, :].rearrange(
        "p (h g dd) -> p h g dd", h=4, g=4)[:, hbase:hbase + 2, :, 0:D]
    nc.vector.tensor_tensor(vslice, s_v, scale_v, op=mybir.AluOpType.mult)
    for h in (hbase, hbase + 1):
        tp = psum_tp.tile([128, B], F32, tag="tp")
        in_slice = vpad[:, h * 128:(h + 1) * 128]
        nc.tensor.transpose(tp[:, :], in_slice, ident[:, :])
        nc.scalar.copy(vT[:, h * B:(h + 1) * B], tp[:, :])


def _emit_iteration_ul(nc, tc, pools, vT, l_buf, delta_buf, wdt, xbf, itr):
    """u = v.W (PE, per-ih psum banks) -> evac (ACT) -> xu = x*u (DVE) ->
    e-reduction rounds (DVE) -> l (or delta for iter 3). ih-OUTER so the
    softmax/xc/s stage for ih=0 can overlap the ih=1 chains."""
    pool, psum_u, psum_tp, seq = pools
    for ih in range(IH):
        for o in range(O):
            h, g = o // 4, o % 4
            u_ps = psum_u.tile([IL, E * B], F32, tag="u")
            for e in range(E):
                lhsT = wdt[:, :].rearrange(
                    "p (h ih e il) -> p h ih e il", h=4, ih=IH, e=E
                )[32 * g:32 * (g + 1), h, ih, e, :]
                rhs = vT[32 * g:32 * (g + 1), h * B:(h + 1) * B]
                nc.tensor.matmul(
                    u_ps[:, e * B:(e + 1) * B], lhsT, rhs,
                    start=True, stop=True, tile_position=(32 * g, 0),
                )
            u_sb = pool.tile([IL, E * B], BF16, tag="u_sb")
            nc.scalar.copy(u_sb[:, :], u_ps[:, :])
            xu = pool.tile([IL, E * B], BF16, tag="xu")
            nc.vector.tensor_tensor(
                xu[:, :],
                xbf.rearrange("p (ih x) -> p ih x", ih=IH)[:, ih],
                u_sb[:, :], op=mybir.AluOpType.mult,
            )
            r1 = pool.tile([IL, 4 * B], BF16, tag="r1")
            nc.vector.tensor_tensor(r1[:, :], xu[:, 0:4 * B],
                                    xu[:, 4 * B:8 * B],
                                    op=mybir.AluOpType.add)
            r2 = pool.tile([IL, 2 * B], BF16, tag="r2")
            nc.vector.tensor_tensor(r2[:, :], r1[:, 0:2 * B],
                                    r1[:, 2 * B:4 * B],
                                    op=mybir.AluOpType.add)
            dst_buf = l_buf if itr == 2 else delta_buf
            dst = dst_buf[:, :].rearrange(
                "p (o ih b) -> p o ih b", o=O, ih=IH)[:, o, ih]
            nc.vector.tensor_tensor(dst, r2[:, 0:B], r2[:, B:2 * B],
                                    op=mybir.AluOpType.add)
        if itr == 3:
            # fold the agreement delta in per ih-half (overlaps other half)
            lv = l_buf[:, :].rearrange("p (o ih b) -> p o ih b",
                                       o=O, ih=IH)[:, :, ih]
            dv = delta_buf[:, :].rearrange("p (o ih b) -> p o ih b",
                                           o=O, ih=IH)[:, :, ih]
            nc.vector.tensor_tensor(lv, lv, dv, op=mybir.AluOpType.add)


def _emit_softmax_xc_s(nc, tc, pools, l_buf, xbf, wbf, s_ps, itr):
    """Per ih-half: exp (ACT), Z over o (DVE bf16 tree), 1/Z, xp = x/Z,
    then per-o xc = exp*xp and 8 accumulating s-matmuls into s_ps
    [(b), (o,d)=256]. The ih=0 half depends only on l[.,.,0,.] so it
    overlaps the ih=1 ul chains. s_ps is pre-zeroed and accumulated with
    start=False (groups interleave across ih halves)."""
    pool, psum_u, psum_tp, seq = pools
    nc.scalar.memzero(s_ps[:, :])
    exp_buf = seq.tile([IL, O * IH * B], BF16, tag="exp")
    lowp = nc.allow_low_precision(
        reason="softmax Z partial sums in bf16; rel budget 2e-2")
    lowp.__enter__()
    for ih in range(IH):
        l_ih = l_buf[:, :].rearrange("p (o ih b) -> p o ih b",
                                     o=O, ih=IH)[:, :, ih]
        e_ih = exp_buf[:, :].rearrange("p (o ih b) -> p o ih b",
                                       o=O, ih=IH)[:, :, ih]
        nc.scalar.activation(e_ih, l_ih, mybir.ActivationFunctionType.Exp)
        za1 = seq.tile([IL, 8 * B], BF16, tag=f"za1{ih}")
        nc.vector.tensor_add(
            za1[:, :].rearrange("p (o b) -> p o b", o=8),
            e_ih[:, 0:8], e_ih[:, 8:16])
        za2 = seq.tile([IL, 4 * B], BF16, tag=f"za2{ih}")
        nc.vector.tensor_add(za2[:, :], za1[:, 0:4 * B], za1[:, 4 * B:])
        za3 = seq.tile([IL, 2 * B], BF16, tag=f"za3{ih}")
        nc.vector.tensor_add(za3[:, :], za2[:, 0:2 * B], za2[:, 2 * B:])
        zbuf = seq.tile([IL, B], F32, tag=f"z{ih}")
        nc.vector.tensor_add(zbuf[:, :], za3[:, 0:B], za3[:, B:])
        rz = seq.tile([IL, B], BF16, tag=f"rz{ih}")
        nc.vector.reciprocal(rz[:, :], zbuf[:, :])
        xp = seq.tile([IL, E * B], BF16, tag=f"xp{ih}")
        nc.vector.tensor_tensor(
            xp[:, :].rearrange("p (e b) -> p e b", e=E),
            xbf.rearrange("p (ih e b) -> p ih e b", ih=IH, e=E)[:, ih],
            rz[:, :].unsqueeze(1).broadcast_to((IL, E, B)),
            op=mybir.AluOpType.mult,
        )
        for o in range(O):
            xc = pool.tile([IL, E * B], BF16, tag="xc")
            nc.vector.tensor_tensor(
                xc[:, :].rearrange("p (e b) -> p e b", e=E),
                e_ih[:, o].unsqueeze(1).broadcast_to((IL, E, B)),
                xp[:, :].rearrange("p (e b) -> p e b", e=E),
                op=mybir.AluOpType.mult,
            )
            for e in range(E):
                lhsT = xc[:, e * B:(e + 1) * B]
                rhs = wbf.rearrange(
                    "p (ih e o d) -> p ih e (o d)", ih=IH, e=E, o=O
                )[:, ih, e, o * D:(o + 1) * D]
                nc.tensor.matmul(
                    s_ps[:, o * D:(o + 1) * D], lhsT, rhs,
                    start=False, stop=False, skip_group_check=True,
                )
    lowp.__exit__(None, None, None)


def build():
    nc = bacc.Bacc("TRN2", target_bir_lowering=False, debug=False,
                   enable_asserts=True, num_devices=N_CORES)

    # per-core inputs (host pre-arranged + I-axis rotated; see kernel())
    xf_d = nc.dram_tensor("xf", [IL, NIH * E * B], F8,
                          kind="ExternalInput").ap()
    wf_d = nc.dram_tensor("wf", [IL, NIH * E * O * D], F8,
                          kind="ExternalInput").ap()
    xbf_d = nc.dram_tensor("xbf", [IL, IH * E * B], BF16,
                           kind="ExternalInput").ap()
    wbf_d = nc.dram_tensor("wbf", [IL, IH * E * O * D], BF16,
                           kind="ExternalInput").ap()
    wdt_d = nc.dram_tensor("wdt", [64, 4 * IH * E * IL], BF16,
                           kind="ExternalInput").ap()
    ident_d = nc.dram_tensor("ident", [128, 128], F32,
                             kind="ExternalInput").ap()
    sp_out = nc.dram_tensor("sp", [B, O * D], F32, kind="ExternalOutput").ap()

    warm_in = nc.dram_tensor("warm_in", [1, 8], F32)
    warm_out = nc.dram_tensor("warm_out", [1, 8], F32, addr_space="Shared")
    cc_in = [nc.dram_tensor(f"cc{i}_in", [B, 8 * D], F32) for i in range(2)]
    cc_out = [nc.dram_tensor(f"cc{i}_out", [B, 8 * D], F32,
                             addr_space="Shared") for i in range(2)]

    rg = [list(range(N_CORES))]

    with tile.TileContext(nc) as tc:
        with (
            tc.tile_pool(name="const", bufs=1) as cpool,
            tc.tile_pool(name="work", bufs=3) as pool,
            tc.tile_pool(name="psum_u", bufs=2, space="PSUM") as psum_u,
            tc.tile_pool(name="psum_s", bufs=2, space="PSUM") as psum_s,
            tc.tile_pool(name="psum_tp", bufs=2, space="PSUM") as psum_tp,
            tc.tile_pool(name="seq", bufs=1) as seq_pool,
        ):
            # warmup collective: pays the one-time collectives init/skew
            # (~35us) fully overlapped with s1 + iter-2 compute, so AR2
            # (the only real collective) starts with ~1us trigger delay.
            warm_sb = cpool.tile([1, 8], F32)
            nc.vector.memset(warm_sb[:, :], 0.0)
            nc.sync.dma_start(out=warm_in[:], in_=warm_sb[:, :])
            nc.gpsimd.collective_compute(
                "AllReduce", mybir.AluOpType.add, replica_groups=rg,
                ins=[warm_in[:]], outs=[warm_out[:]],
            )

            # PE pre-warm: ~3.5us of junk matmuls lifts the HAM clock gate
            # to 8/8 before the s1 chain starts (else s1 runs at 1.2 GHz).
            junk = cpool.tile([128, B], BF16)
            nc.vector.memset(junk[:, :], 0.0)
            for w in range(120):
                wp = psum_tp.tile([128, B], F32, tag="tp")
                nc.tensor.matmul(wp[:, :], junk[:, :], junk[:, :],
                                 start=True, stop=True)

            # ---- load inputs; wf in per-ihf chunks so s1 pipelines ----
            xf = cpool.tile([IL, NIH * E * B], F8)
            nc.sync.dma_start(out=xf[:, :], in_=xf_d)
            ident = cpool.tile([128, 128], F32)
            nc.sync.dma_start(out=ident[:, :], in_=ident_d)
            WCH = E * O * D  # wf chunk cols per ihf block
            wf = cpool.tile([IL, NIH * WCH], F8)
            for ihf in range(NIH):
                nc.sync.dma_start(
                    out=wf[:, ihf * WCH:(ihf + 1) * WCH],
                    in_=wf_d.rearrange(
                        "p (f r) -> p f r", f=NIH)[:, ihf, :])
            xbf_t = cpool.tile([IL, IH * E * B], BF16)
            nc.sync.dma_start(out=xbf_t[:, :], in_=xbf_d)
            wbf_t = cpool.tile([IL, IH * E * O * D], BF16)
            nc.sync.dma_start(out=wbf_t[:, :], in_=wbf_d)
            wdt = cpool.tile([128, 4 * IH * E * IL], BF16)
            nc.vector.memset(wdt[:, :], 0.0)
            for g in range(4):
                nc.sync.dma_start(out=wdt[32 * g:32 * g + D, :],
                                  in_=wdt_d[16 * g:16 * (g + 1), :])

            xbf = xbf_t[:, :]
            wbf = wbf_t[:, :]

            l_buf = cpool.tile([IL, O * IH * B], BF16)
            delta_buf = cpool.tile([IL, O * IH * B], BF16)
            vpad = cpool.tile([B, 4 * 4 * 32], F32)
            nc.vector.memset(vpad[:, :], 0.0)
            vT = cpool.tile([128, 4 * B], BF16)

            pools = (pool, psum_u, psum_tp, seq_pool)

            # ---- iteration 1: full s1 = (1/16) x @ W locally, no AR ----
            _sid_s1, _ = nc.enter_named_scope("s1", False)
            s_ps1 = psum_s.tile([B, O * D], F32, tag="s")
            kt = 0
            for ihf in range(NIH):
                for e in range(E):
                    lhsT = xf[:, :].rearrange(
                        "p (f e b) -> p f e b", f=NIH, e=E)[:, ihf, e, :]
                    rhs = wf[:, :].rearrange(
                        "p (f e od) -> p f e od", f=NIH, e=E)[:, ihf, e, :]
                    nc.tensor.matmul(
                        s_ps1[:, :], lhsT, rhs,
                        start=(kt == 0), stop=(kt == NIH * E - 1),
                    )
                    kt += 1
            s_sb1 = cpool.tile([B, O * D], F32)
            nc.scalar.mul(s_sb1[:, :], s_ps1[:, :], 1.0 / (O * WS))
            nc.leave_named_scope("s1", _sid_s1, False)
            _sid_sq1, _ = nc.enter_named_scope("squash1", False)
            for hb in (0, 2):
                _emit_squash_half(nc, cpool, s_sb1[:, 128 * (hb // 2):
                                                   128 * (hb // 2 + 1)],
                                  vpad, psum_tp, vT, ident, hb, f"1{hb}")
            nc.leave_named_scope("squash1", _sid_sq1, False)

            # ---- iteration 2 ----
            _sid_ul2, _ = nc.enter_named_scope("ul2", False)
            _emit_iteration_ul(nc, tc, pools, vT, l_buf, delta_buf, wdt,
                               xbf, 2)
            nc.leave_named_scope("ul2", _sid_ul2, False)
            _sid_xcs2, _ = nc.enter_named_scope("xcs2", False)
            s_ps2 = psum_s.tile([B, O * D], F32, tag="s")
            _emit_softmax_xc_s(nc, tc, pools, l_buf, xbf, wbf, s_ps2, 2)
            s_sb2 = [cpool.tile([B, 8 * D], F32, tag=f"ssb{i}",
                                name=f"ssb{i}") for i in range(2)]
            for half in range(2):
                nc.scalar.copy(s_sb2[half][:, :],
                               s_ps2[:, 128 * half:128 * (half + 1)])
                nc.sync.dma_start(out=cc_in[half][:],
                                  in_=s_sb2[half][:, :])
            nc.leave_named_scope("xcs2", _sid_xcs2, False)

            _sid_ar2, _ = nc.enter_named_scope("ar2", False)
            sfull2 = [cpool.tile([B, 8 * D], F32, tag=f"sf{i}",
                                 name=f"sf{i}") for i in range(2)]
            for half in range(2):
                nc.gpsimd.collective_compute(
                    "AllReduce", mybir.AluOpType.add, replica_groups=rg,
                    ins=[cc_in[half][:]], outs=[cc_out[half][:]],
                )
                nc.sync.dma_start(out=sfull2[half][:, :],
                                  in_=cc_out[half][:])
                _emit_squash_half(nc, cpool, sfull2[half], vpad, psum_tp,
                                  vT, ident, 2 * half, f"2{half}")
            nc.leave_named_scope("ar2", _sid_ar2, False)

            # ---- iteration 3 (final: partial s3 out, host finishes) ----
            _sid_ul3, _ = nc.enter_named_scope("ul3", False)
            _emit_iteration_ul(nc, tc, pools, vT, l_buf, delta_buf, wdt,
                               xbf, 3)
            nc.leave_named_scope("ul3", _sid_ul3, False)
            _sid_xcs3, _ = nc.enter_named_scope("xcs3", False)
            s_ps3 = psum_s.tile([B, O * D], F32, tag="s")
            _emit_softmax_xc_s(nc, tc, pools, l_buf, xbf, wbf, s_ps3, 3)
            nc.leave_named_scope("xcs3", _sid_xcs3, False)
            sp_sb = cpool.tile([B, O * D], F32)
            nc.scalar.copy(sp_sb[:, :], s_ps3[:, :])
            nc.sync.dma_start(out=sp_out, in_=sp_sb[:, :])

    nc.compile()
    return nc


def _host_prep(x, weight):
    """Per-core input maps. The I axis is rotated per core so that the
    core's own slice occupies blocks 0..1 of the full tensors."""
    in_maps = []
    ident = np.eye(128, dtype=np.float32)
    for c in range(N_CORES):
        order = np.concatenate([
            np.arange(c * IC, (c + 1) * IC),
            np.arange(0, c * IC),
            np.arange((c + 1) * IC, I_FULL),
        ])
        x_r = x[:, order, :]          # [B, I, E]
        w_r = weight[:, order, :, :]  # [O, I, D, E]

        # xf [il, (ihf, e, b)] fp8 (s1 only)
        xr = x_r.reshape(B, NIH, IL, E)
        xf = np.ascontiguousarray(
            xr.transpose(2, 1, 3, 0)              # il, ihf, e, b
        ).reshape(IL, NIH * E * B)

        # wf [il, (ihf, e, o, d)] fp8, pre-scaled by WS (s1 only)
        wr = w_r.reshape(O, NIH, IL, D, E)
        wfull = np.ascontiguousarray(
            wr.transpose(2, 1, 4, 0, 3)           # il, ihf, e, o, d
        ).reshape(IL, NIH * E * O * D)

        # bf16 slice tensors for the iterations (own slice = blocks 0..1)
        xsl = xf[:, 0:IH * E * B]
        wsl = wfull[:, 0:IH * E * O * D]

        # wdt compact [(g, d=16), (h, ih, e, il)] from own slice, o = 4h+g
        w_own = w_r[:, 0:IC]                      # [O, 256, D, E]
        wo = w_own.reshape(4, 4, IH, IL, D, E)    # h, g, ih, il, d, e
        wdt = np.ascontiguousarray(
            wo.transpose(1, 4, 0, 2, 5, 3)        # g, d, h, ih, e, il
        ).reshape(64, 4 * IH * E * IL)

        in_maps.append({
            "xf": xf.astype(NPF8),
            "wf": (wfull * WS).astype(NPF8),
            "xbf": xsl.astype(NPBF16),
            "wbf": wsl.astype(NPBF16),
            "wdt": wdt.astype(NPBF16),
            "ident": ident,
        })
    return in_maps


def _host_finish(partials):
    """Sum the 8 per-core partial s3 tensors, final squash (the unshard)."""
    s = np.zeros((B, O * D), dtype=np.float64)
    for p in partials:
        s += p.astype(np.float64)
    s = s.reshape(B, O, D)
    n2 = (s * s).sum(axis=-1, keepdims=True)
    n = np.sqrt(n2)
    v = (n2 / (1.0 + n2) / (n + EPS)) * s
    return v.astype(np.float32)


def kernel(x, weight, _trace=False):
    x = np.asarray(x, dtype=np.float32)
    weight = np.asarray(weight, dtype=np.float32)
    if "nc" not in _CACHE:
        _CACHE["nc"] = build()
    nc = _CACHE["nc"]
    in_maps = _host_prep(x, weight)
    res = run_bass_kernel_spmd(
        nc, in_maps, core_ids=list(range(N_CORES)), trace=_trace
    )
    out = _host_finish([res.results[c]["sp"] for c in range(N_CORES)])
    if _trace:
        _CACHE["last_result"] = res
    return out


if __name__ == "__main__":
    rng = np.random.default_rng(0)
    x = rng.standard_normal((B, I_FULL, E)).astype(np.float32)
    w = (0.01 * rng.standard_normal((O, I_FULL, D, E))).astype(np.float32)
    out = kernel(x, w)
    print("out", out.shape, out.dtype, np.abs(out).max())


# revision 55
# speedup vs baseline: 1.1750x; 1.1750x over previous
"""Trainium2 Bass kernel for DenseCapsule dynamic routing (3 iterations).

Problem: x[128,2048,8] f32, weight[16,2048,16,8] f32 -> out[128,16,16] f32.
  x_hat = einsum('oide,bie->boid', W, x); 3 routing iterations
  (softmax over o, c-weighted i-sum, squash, agreement update).

Strategy (8 NeuronCores, shard in_num_caps I=2048 -> 256 per core):
  x_hat is never materialized. Per iteration, everything factors through W:
    u = v . W (PE), l = sum_e x*u (DVE), softmax (ACT/DVE),
    xc = c*x (DVE), s = xc @ W (PE).

  v4: iteration 1's c is UNIFORM, so s1 = (1/16) sum_i xhat is fully
  input-determined. Every core loads the FULL W (bf16, host-rotated so its
  own I-slice sits in blocks 0..1) and computes the full s1 locally with
  one 128-matmul chain (overlapped with the W DMA stream) -> NO AllReduce
  for iteration 1. The collectives runtime's ~38us barrier + ~36us
  first-collective init run in the background (~21us in, time-based) and
  are fully amortized by the time AR2 (the only collective) triggers.
  AR2 is split into o-halves so squash/transpose/ul3 of the first half
  overlap the second half's AllReduce.

Layout conventions per core (SBUF partition dim first):
  i_local = ihf*128 + il over the ROTATED I axis (own slice = ihf 0..1)
  o = 4*h + g            (g in 0..3 selects a 32-partition group, h in 0..3)
  d padded to 32 rows (dd) for the u-matmul stationary operand.
"""

import sys

for _p in ("/opt/trn_rl_repo", "/root/.axon_site/_ro/trn_rl_repo"):
    if _p not in sys.path:
        sys.path.insert(0, _p)

import numpy as np
import ml_dtypes

import concourse.bass as bass
import concourse.bacc as bacc
import concourse.mybir as mybir
import concourse.tile as tile
from concourse.bass_utils import run_bass_kernel_spmd

F32 = mybir.dt.float32
BF16 = mybir.dt.bfloat16
F8 = mybir.dt.float8e4
NPBF16 = ml_dtypes.bfloat16
NPF8 = ml_dtypes.float8_e4m3
WS = 64.0  # host pre-scale on fp8 W (keeps values out of fp8 subnormals)

N_CORES = 8
B = 128          # batch
I_FULL = 2048    # in caps
IC = 256         # in caps per core
IL = 128         # partition dim of i
IH = IC // IL    # 2 (own slice blocks)
NIH = I_FULL // IL  # 16 (full-I blocks, rotated: 0..1 are "ours")
E = 8            # in cap dim
O = 16           # out caps
D = 16           # out cap dim
EPS = 1e-8

_CACHE = {}


def _emit_squash_half(nc, pool, sfull_h, vpad, psum_tp, vT, ident, hbase,
                      tag):
    """squash on one o-half: sfull_h [(b)=128, (2h,g,d)=128] f32 covering
    o = 4*hbase .. 4*hbase+7; writes v into vpad cols for h = hbase,
    hbase+1 and produces the two vT column blocks via PE transposes."""
    HO = 8  # o's in this half
    sq = pool.tile([B, HO * D], F32, tag=f"sq{tag}")
    nc.scalar.square(sq[:, :], sfull_h[:, :])
    nrm2 = pool.tile([B, HO], F32, tag=f"nrm2{tag}")
    nc.vector.reduce_sum(
        nrm2[:, :],
        sq[:, :].rearrange("p (o d) -> p o d", d=D),
        axis=mybir.AxisListType.X,
    )
    q = pool.tile([B, HO], F32, tag=f"q{tag}")
    nc.scalar.sqrt(q[:, :], nrm2[:, :])
    t1 = pool.tile([B, HO], F32, tag=f"t1{tag}")
    nc.vector.tensor_scalar_add(t1[:, :], nrm2[:, :], 1.0)
    t2 = pool.tile([B, HO], F32, tag=f"t2{tag}")
    nc.vector.tensor_scalar_add(t2[:, :], q[:, :], EPS)
    den = pool.tile([B, HO], F32, tag=f"den{tag}")
    nc.vector.tensor_mul(den[:, :], t1[:, :], t2[:, :])
    rden = pool.tile([B, HO], F32, tag=f"rden{tag}")
    nc.vector.reciprocal(rden[:, :], den[:, :])
    scale = pool.tile([B, HO], F32, tag=f"scale{tag}")
    nc.vector.tensor_mul(scale[:, :], nrm2[:, :], rden[:, :])
    # v = s * scale (broadcast over d) into vpad[(b), (hh, g, dd<16)]
    s_v = sfull_h[:, :].rearrange("p (hh g d) -> p hh g d", hh=2, g=4)
    scale_v = scale[:, :].rearrange("p (hh g) -> p hh g", hh=2).broadcast_to(
        (B, 2, 4, D)
    )
    vslice = vpad[:, :].rearrange(
        "p (h g dd) -> p h g dd", h=4, g=4)[:, hbase:hbase + 2, :, 0:D]
    nc.vector.tensor_tensor(vslice, s_v, scale_v, op=mybir.AluOpType.mult)
    for h in (hbase, hbase + 1):
        tp = psum_tp.tile([128, B], F32, tag="tp")
        in_slice = vpad[:, h * 128:(h + 1) * 128]
        nc.tensor.transpose(tp[:, :], in_slice, ident[:, :])
        nc.scalar.copy(vT[:, h * B:(h + 1) * B], tp[:, :])


def _emit_iteration_ul(nc, tc, pools, vT, l_buf, delta_buf, wdt, xbf, itr):
    """u = v.W (PE, per-ih psum banks) -> evac (ACT) -> xu = x*u (DVE) ->
    e-reduction rounds (DVE) -> l (or delta for iter 3)."""
    pool, psum_u, psum_tp, seq = pools
    for o in range(O):
        h, g = o // 4, o % 4
        u_sb = pool.tile([IL, IH * E * B], BF16, tag="u_sb")
        for ih in range(IH):
            u_ps = psum_u.tile([IL, E * B], F32, tag="u")
            for e in range(E):
                lhsT = wdt[:, :].rearrange(
                    "p (h ih e il) -> p h ih e il", h=4, ih=IH, e=E
                )[32 * g:32 * (g + 1), h, ih, e, :]
                rhs = vT[32 * g:32 * (g + 1), h * B:(h + 1) * B]
                nc.tensor.matmul(
                    u_ps[:, e * B:(e + 1) * B], lhsT, rhs,
                    start=True, stop=True, tile_position=(32 * g, 0),
                )
            nc.scalar.copy(u_sb[:, ih * E * B:(ih + 1) * E * B], u_ps[:, :])
        xu = pool.tile([IL, IH * E * B], BF16, tag="xu")
        nc.vector.tensor_tensor(
            xu[:, :], xbf, u_sb[:, :], op=mybir.AluOpType.mult,
        )
        xu4 = xu[:, :].rearrange("p (ih half eb) -> p ih half eb",
                                 ih=IH, half=2)
        r1 = pool.tile([IL, IH * 4 * B], BF16, tag="r1")
        r1v = r1[:, :].rearrange("p (ih eb) -> p ih eb", ih=IH)
        nc.vector.tensor_tensor(r1v, xu4[:, :, 0], xu4[:, :, 1],
                                op=mybir.AluOpType.add)
        r1h = r1[:, :].rearrange("p (ih half eb) -> p ih half eb",
                                 ih=IH, half=2)
        r2 = pool.tile([IL, IH * 2 * B], BF16, tag="r2")
        r2v = r2[:, :].rearrange("p (ih eb) -> p ih eb", ih=IH)
        nc.vector.tensor_tensor(r2v, r1h[:, :, 0], r1h[:, :, 1],
                                op=mybir.AluOpType.add)
        r2h = r2[:, :].rearrange("p (ih half b) -> p ih half b",
                                 ih=IH, half=2)
        dst_buf = l_buf if itr == 2 else delta_buf
        dst = dst_buf[:, :].rearrange(
            "p (o ih b) -> p o ih b", o=O, ih=IH
        )[:, o]
        nc.vector.tensor_tensor(dst, r2h[:, :, 0], r2h[:, :, 1],
                                op=mybir.AluOpType.add)
    if itr == 3:
        HB = O * IH * B // 2
        nc.vector.tensor_add(l_buf[:, 0:HB], l_buf[:, 0:HB],
                             delta_buf[:, 0:HB])
        nc.vector.tensor_add(l_buf[:, HB:], l_buf[:, HB:],
                             delta_buf[:, HB:])


def _emit_softmax_xc_s(nc, tc, pools, l_buf, xbf, wbf, s_ps, itr):
    """exp (ACT), Z (DVE bf16 trees), 1/Z, xp = x/Z, then per-o xc = exp*xp
    and the 16 accumulating s-matmuls into s_ps [(b), (o,d)=256]."""
    pool, psum_u, psum_tp, seq = pools
    exp_buf = seq.tile([IL, O * IH * B], BF16, tag="exp")
    HALF = 8 * IH * B
    lowp = nc.allow_low_precision(
        reason="softmax Z partial sums in bf16; rel budget 2e-2")
    lowp.__enter__()
    nc.scalar.activation(
        exp_buf[:, 0:HALF], l_buf[:, 0:HALF],
        mybir.ActivationFunctionType.Exp)
    za1 = seq.tile([IL, 4 * IH * B], BF16, tag="za1")
    nc.vector.tensor_add(za1[:, :], exp_buf[:, 0:HALF // 2],
                         exp_buf[:, HALF // 2:HALF])
    za2 = seq.tile([IL, 2 * IH * B], BF16, tag="za2")
    nc.vector.tensor_add(za2[:, :], za1[:, 0:2 * IH * B],
                         za1[:, 2 * IH * B:4 * IH * B])
    za3 = seq.tile([IL, IH * B], BF16, tag="za3")
    nc.vector.tensor_add(za3[:, :], za2[:, 0:IH * B],
                         za2[:, IH * B:2 * IH * B])
    nc.scalar.activation(
        exp_buf[:, HALF:2 * HALF], l_buf[:, HALF:2 * HALF],
        mybir.ActivationFunctionType.Exp)
    zb1 = seq.tile([IL, 4 * IH * B], BF16, tag="zb1")
    nc.vector.tensor_add(zb1[:, :], exp_buf[:, HALF:HALF + HALF // 2],
                         exp_buf[:, HALF + HALF // 2:2 * HALF])
    zb2 = seq.tile([IL, 2 * IH * B], BF16, tag="zb2")
    nc.vector.tensor_add(zb2[:, :], zb1[:, 0:2 * IH * B],
                         zb1[:, 2 * IH * B:4 * IH * B])
    zb3 = seq.tile([IL, IH * B], BF16, tag="zb3")
    nc.vector.tensor_add(zb3[:, :], zb2[:, 0:IH * B],
                         zb2[:, IH * B:2 * IH * B])
    zbuf = seq.tile([IL, IH * B], F32, tag="z")
    nc.vector.tensor_add(zbuf[:, :], za3[:, :], zb3[:, :])
    rz = seq.tile([IL, IH * B], BF16, tag="rz")
    nc.vector.reciprocal(rz[:, :], zbuf[:, :])
    lowp.__exit__(None, None, None)
    xp = seq.tile([IL, IH * E * B], BF16, tag="xp")
    nc.vector.tensor_tensor(
        xp[:, :].rearrange("p (ih e b) -> p ih e b", ih=IH, e=E),
        xbf.rearrange("p (ih e b) -> p ih e b", ih=IH, e=E),
        rz[:, :].rearrange("p (ih b) -> p ih b", ih=IH)
        .unsqueeze(2).broadcast_to((IL, IH, E, B)),
        op=mybir.AluOpType.mult,
    )
    for o in range(O):
        xc = pool.tile([IL, IH * E * B], BF16, tag="xc")
        nc.vector.tensor_tensor(
            xc[:, :].rearrange("p (ih e b) -> p ih e b", ih=IH, e=E),
            exp_buf[:, :].rearrange("p (o ih b) -> p o ih b", o=O, ih=IH)[:, o]
            .unsqueeze(2).broadcast_to((IL, IH, E, B)),
            xp[:, :].rearrange("p (ih e b) -> p ih e b", ih=IH, e=E),
            op=mybir.AluOpType.mult,
        )
        n_k = IH * E
        kt = 0
        for ih in range(IH):
            for e in range(E):
                lhsT = xc[:, :].rearrange(
                    "p (ih e b) -> p ih e b", ih=IH, e=E
                )[:, ih, e, :]
                rhs = wbf.rearrange(
                    "p (ih e o d) -> p ih e (o d)", ih=IH, e=E, o=O
                )[:, ih, e, o * D:(o + 1) * D]
                nc.tensor.matmul(
                    s_ps[:, o * D:(o + 1) * D], lhsT, rhs,
                    start=(kt == 0), stop=(kt == n_k - 1),
                )
                kt += 1


def build():
    nc = bacc.Bacc("TRN2", target_bir_lowering=False, debug=False,
                   enable_asserts=True, num_devices=N_CORES)

    # per-core inputs (host pre-arranged + I-axis rotated; see kernel())
    xf_d = nc.dram_tensor("xf", [IL, NIH * E * B], F8,
                          kind="ExternalInput").ap()
    wf_d = nc.dram_tensor("wf", [IL, NIH * E * O * D], F8,
                          kind="ExternalInput").ap()
    xbf_d = nc.dram_tensor("xbf", [IL, IH * E * B], BF16,
                           kind="ExternalInput").ap()
    wbf_d = nc.dram_tensor("wbf", [IL, IH * E * O * D], BF16,
                           kind="ExternalInput").ap()
    wdt_d = nc.dram_tensor("wdt", [64, 4 * IH * E * IL], BF16,
                           kind="ExternalInput").ap()
    ident_d = nc.dram_tensor("ident", [128, 128], F32,
                             kind="ExternalInput").ap()
    sp_out = nc.dram_tensor("sp", [B, O * D], F32, kind="ExternalOutput").ap()

    warm_in = nc.dram_tensor("warm_in", [1, 8], F32)
    warm_out = nc.dram_tensor("warm_out", [1, 8], F32, addr_space="Shared")
    cc_in = [nc.dram_tensor(f"cc{i}_in", [B, 8 * D], F32) for i in range(2)]
    cc_out = [nc.dram_tensor(f"cc{i}_out", [B, 8 * D], F32,
                             addr_space="Shared") for i in range(2)]

    rg = [list(range(N_CORES))]

    with tile.TileContext(nc) as tc:
        with (
            tc.tile_pool(name="const", bufs=1) as cpool,
            tc.tile_pool(name="work", bufs=3) as pool,
            tc.tile_pool(name="psum_u", bufs=2, space="PSUM") as psum_u,
            tc.tile_pool(name="psum_s", bufs=2, space="PSUM") as psum_s,
            tc.tile_pool(name="psum_tp", bufs=2, space="PSUM") as psum_tp,
            tc.tile_pool(name="seq", bufs=1) as seq_pool,
        ):
            # warmup collective: pays the one-time collectives init/skew
            # (~35us) fully overlapped with s1 + iter-2 compute, so AR2
            # (the only real collective) starts with ~1us trigger delay.
            warm_sb = cpool.tile([1, 8], F32)
            nc.vector.memset(warm_sb[:, :], 0.0)
            nc.sync.dma_start(out=warm_in[:], in_=warm_sb[:, :])
            nc.gpsimd.collective_compute(
                "AllReduce", mybir.AluOpType.add, replica_groups=rg,
                ins=[warm_in[:]], outs=[warm_out[:]],
            )

            # PE pre-warm: ~3.5us of junk matmuls lifts the HAM clock gate
            # to 8/8 before the s1 chain starts (else s1 runs at 1.2 GHz).
            junk = cpool.tile([128, B], BF16)
            nc.vector.memset(junk[:, :], 0.0)
            for w in range(120):
                wp = psum_tp.tile([128, B], F32, tag="tp")
                nc.tensor.matmul(wp[:, :], junk[:, :], junk[:, :],
                                 start=True, stop=True)

            # ---- load inputs; wf in per-ihf chunks so s1 pipelines ----
            xf = cpool.tile([IL, NIH * E * B], F8)
            nc.sync.dma_start(out=xf[:, :], in_=xf_d)
            ident = cpool.tile([128, 128], F32)
            nc.sync.dma_start(out=ident[:, :], in_=ident_d)
            WCH = E * O * D  # wf chunk cols per ihf block
            wf = cpool.tile([IL, NIH * WCH], F8)
            for ihf in range(NIH):
                nc.sync.dma_start(
                    out=wf[:, ihf * WCH:(ihf + 1) * WCH],
                    in_=wf_d.rearrange(
                        "p (f r) -> p f r", f=NIH)[:, ihf, :])
            xbf_t = cpool.tile([IL, IH * E * B], BF16)
            nc.sync.dma_start(out=xbf_t[:, :], in_=xbf_d)
            wbf_t = cpool.tile([IL, IH * E * O * D], BF16)
            nc.sync.dma_start(out=wbf_t[:, :], in_=wbf_d)
            wdt = cpool.tile([128, 4 * IH * E * IL], BF16)
            nc.vector.memset(wdt[:, :], 0.0)
            for g in range(4):
                nc.sync.dma_start(out=wdt[32 * g:32 * g + D, :],
                                  in_=wdt_d[16 * g:16 * (g + 1), :])

            xbf = xbf_t[:, :]
            wbf = wbf_t[:, :]

            l_buf = cpool.tile([IL, O * IH * B], BF16)
            delta_buf = cpool.tile([IL, O * IH * B], BF16)
            vpad = cpool.tile([B, 4 * 4 * 32], F32)
            nc.vector.memset(vpad[:, :], 0.0)
            vT = cpool.tile([128, 4 * B], BF16)

            pools = (pool, psum_u, psum_tp, seq_pool)

            # ---- iteration 1: full s1 = (1/16) x @ W locally, no AR ----
            _sid_s1, _ = nc.enter_named_scope("s1", False)
            s_ps1 = psum_s.tile([B, O * D], F32, tag="s")
            kt = 0
            for ihf in range(NIH):
                for e in range(E):
                    lhsT = xf[:, :].rearrange(
                        "p (f e b) -> p f e b", f=NIH, e=E)[:, ihf, e, :]
                    rhs = wf[:, :].rearrange(
                        "p (f e od) -> p f e od", f=NIH, e=E)[:, ihf, e, :]
                    nc.tensor.matmul(
                        s_ps1[:, :], lhsT, rhs,
                        start=(kt == 0), stop=(kt == NIH * E - 1),
                    )
                    kt += 1
            s_sb1 = cpool.tile([B, O * D], F32)
            nc.scalar.mul(s_sb1[:, :], s_ps1[:, :], 1.0 / (O * WS))
            nc.leave_named_scope("s1", _sid_s1, False)
            _sid_sq1, _ = nc.enter_named_scope("squash1", False)
            for hb in (0, 2):
                _emit_squash_half(nc, cpool, s_sb1[:, 128 * (hb // 2):
                                                   128 * (hb // 2 + 1)],
                                  vpad, psum_tp, vT, ident, hb, f"1{hb}")
            nc.leave_named_scope("squash1", _sid_sq1, False)

            # ---- iteration 2 ----
            _sid_ul2, _ = nc.enter_named_scope("ul2", False)
            _emit_iteration_ul(nc, tc, pools, vT, l_buf, delta_buf, wdt,
                               xbf, 2)
            nc.leave_named_scope("ul2", _sid_ul2, False)
            _sid_xcs2, _ = nc.enter_named_scope("xcs2", False)
            s_ps2 = psum_s.tile([B, O * D], F32, tag="s")
            _emit_softmax_xc_s(nc, tc, pools, l_buf, xbf, wbf, s_ps2, 2)
            s_sb2 = [cpool.tile([B, 8 * D], F32, tag=f"ssb{i}",
                                name=f"ssb{i}") for i in range(2)]
            for half in range(2):
                nc.scalar.copy(s_sb2[half][:, :],
                               s_ps2[:, 128 * half:128 * (half + 1)])
                nc.sync.dma_start(out=cc_in[half][:],
                                  in_=s_sb2[half][:, :])
            nc.leave_named_scope("xcs2", _sid_xcs2, False)

            _sid_ar2, _ = nc.enter_named_scope("ar2", False)
            sfull2 = [cpool.tile([B, 8 * D], F32, tag=f"sf{i}",
                                 name=f"sf{i}") for i in range(2)]
            for half in range(2):
                nc.gpsimd.collective_compute(
                    "AllReduce", mybir.AluOpType.add, replica_groups=rg,
                    ins=[cc_in[half][:]], outs=[cc_out[half][:]],
                )
                nc.sync.dma_start(out=sfull2[half][:, :],
                                  in_=cc_out[half][:])
                _emit_squash_half(nc, cpool, sfull2[half], vpad, psum_tp,
                                  vT, ident, 2 * half, f"2{half}")
            nc.leave_named_scope("ar2", _sid_ar2, False)

            # ---- iteration 3 (final: partial s3 out, host finishes) ----
            _sid_ul3, _ = nc.enter_named_scope("ul3", False)
            _emit_iteration_ul(nc, tc, pools, vT, l_buf, delta_buf, wdt,
                               xbf, 3)
            nc.leave_named_scope("ul3", _sid_ul3, False)
            _sid_xcs3, _ = nc.enter_named_scope("xcs3", False)
            s_ps3 = psum_s.tile([B, O * D], F32, tag="s")
            _emit_softmax_xc_s(nc, tc, pools, l_buf, xbf, wbf, s_ps3, 3)
            nc.leave_named_scope("xcs3", _sid_xcs3, False)
            sp_sb = cpool.tile([B, O * D], F32)
            nc.scalar.copy(sp_sb[:, :], s_ps3[:, :])
            nc.sync.dma_start(out=sp_out, in_=sp_sb[:, :])

    nc.compile()
    return nc


def _host_prep(x, weight):
    """Per-core input maps. The I axis is rotated per core so that the
    core's own slice occupies blocks 0..1 of the full tensors."""
    in_maps = []
    ident = np.eye(128, dtype=np.float32)
    for c in range(N_CORES):
        order = np.concatenate([
            np.arange(c * IC, (c + 1) * IC),
            np.arange(0, c * IC),
            np.arange((c + 1) * IC, I_FULL),
        ])
        x_r = x[:, order, :]          # [B, I, E]
        w_r = weight[:, order, :, :]  # [O, I, D, E]

        # xf [il, (ihf, e, b)] fp8 (s1 only)
        xr = x_r.reshape(B, NIH, IL, E)
        xf = np.ascontiguousarray(
            xr.transpose(2, 1, 3, 0)              # il, ihf, e, b
        ).reshape(IL, NIH * E * B)

        # wf [il, (ihf, e, o, d)] fp8, pre-scaled by WS (s1 only)
        wr = w_r.reshape(O, NIH, IL, D, E)
        wfull = np.ascontiguousarray(
            wr.transpose(2, 1, 4, 0, 3)           # il, ihf, e, o, d
        ).reshape(IL, NIH * E * O * D)

        # bf16 slice tensors for the iterations (own slice = blocks 0..1)
        xsl = xf[:, 0:IH * E * B]
        wsl = wfull[:, 0:IH * E * O * D]

        # wdt compact [(g, d=16), (h, ih, e, il)] from own slice, o = 4h+g
        w_own = w_r[:, 0:IC]                      # [O, 256, D, E]
        wo = w_own.reshape(4, 4, IH, IL, D, E)    # h, g, ih, il, d, e
        wdt = np.ascontiguousarray(
            wo.transpose(1, 4, 0, 2, 5, 3)        # g, d, h, ih, e, il
        ).reshape(64, 4 * IH * E * IL)

        in_maps.append({
            "xf": xf.astype(NPF8),
            "wf": (wfull * WS).astype(NPF8),
            "xbf": xsl.astype(NPBF16),
            "wbf": wsl.astype(NPBF16),
            "wdt": wdt.astype(NPBF16),
            "ident": ident,
        })
    return in_maps


def _host_finish(partials):
    """Sum the 8 per-core partial s3 tensors, final squash (the unshard)."""
    s = np.zeros((B, O * D), dtype=np.float64)
    for p in partials:
        s += p.astype(np.float64)
    s = s.reshape(B, O, D)
    n2 = (s * s).sum(axis=-1, keepdims=True)
    n = np.sqrt(n2)
    v = (n2 / (1.0 + n2) / (n + EPS)) * s
    return v.astype(np.float32)


def kernel(x, weight, _trace=False):
    x = np.asarray(x, dtype=np.float32)
    weight = np.asarray(weight, dtype=np.float32)
    if "nc" not in _CACHE:
        _CACHE["nc"] = build()
    nc = _CACHE["nc"]
    in_maps = _host_prep(x, weight)
    res = run_bass_kernel_spmd(
        nc, in_maps, core_ids=list(range(N_CORES)), trace=_trace
    )
    out = _host_finish([res.results[c]["sp"] for c in range(N_CORES)])
    if _trace:
        _CACHE["last_result"] = res
    return out


if __name__ == "__main__":
    rng = np.random.default_rng(0)
    x = rng.standard_normal((B, I_FULL, E)).astype(np.float32)
    w = (0.01 * rng.standard_normal((O, I_FULL, D, E))).astype(np.float32)
    out = kernel(x, w)
    print("out", out.shape, out.dtype, np.abs(out).max())
